# revision 22
# baseline (speedup 1.0000x reference)
"""MLA (multi-head latent attention) Trainium2 kernel, 8-core SPMD.

Sharding:
  phase 0: DP over sequence  -- each core rmsnorms latents for T/8 tokens
           (computed transposed on-chip), AllGather.
  phase 1: TP over heads     -- 2 heads/core: decompress q/k/v, causal
           attention in S^T [k, q] layout (softmax without max-subtraction;
           row-sums via ones-matmul), AllGather of attn^T.
  phase 2: each core computes its DIM/8 output columns of @ W_o.

Host-side folding: g_q/g_kv and 1/sqrt(head_dim+rope) into up-proj weights;
rope pair de-interleave via weight column permutation; cos/sin shipped
transposed; causal mask via block skip + deduplicated additive mask tiles.
All matmuls in float32r (full-rate fp32 PE mode).
"""

import math
import numpy as np

import concourse.bass as bass
import concourse.tile as tile
from concourse import bacc, mybir
from concourse.bass import ds, ts
from concourse.masks import make_identity

P = 128
F32 = mybir.dt.float32
F32R = mybir.dt.float32r

# problem constants (hardcoded per contract)
DIM = 2048
N_HEADS = 16
HEAD_DIM = 128
Q_C = 1536
KV_C = 512
D_R = 64
B = 1
EPS = 1e-6
N_CORES = 8


def r(ap):
    if ap.dtype != F32R:
        return ap.bitcast(F32R)
    return ap


def _build_block_table(mask, t, q_chunk=512, k_tile=128):
    """Per (q_chunk, k_tile) block: 'skip', ('mask', idx) or 'plain'.
    Mask blocks are deduplicated; returns (table, uniq) where uniq is
    [U, k_tile, q_chunk] additive mask tiles (transposed to [k, q])."""
    uniq = []
    keys = {}
    table = {}
    for qc in range(t // q_chunk):
        q0 = qc * q_chunk
        for kt in range(t // k_tile):
            k0 = kt * k_tile
            blk = mask[q0 : q0 + q_chunk, k0 : k0 + k_tile]  # [q, k]
            if np.all(blk <= -1e8):
                table[(qc, kt)] = ("skip", -1)
            elif np.all(blk == 0.0):
                table[(qc, kt)] = ("plain", -1)
            else:
                bt = np.ascontiguousarray(blk.T)  # [k, q]
                key = bt.tobytes()
                if key not in keys:
                    keys[key] = len(uniq)
                    uniq.append(bt)
                table[(qc, kt)] = ("mask", keys[key])
    if not uniq:
        uniq.append(np.zeros((k_tile, q_chunk), np.float32))
    return table, np.stack(uniq).astype(np.float32)


def build_program(t, block_table, n_mask):
    """Build the SPMD bass program for sequence length t (multiple of 1024)."""
    nc = bacc.Bacc(
        "TRN2", target_bir_lowering=False, debug=False, num_devices=N_CORES
    )
    ts_ = t // N_CORES  # tokens per core in phase 0
    ntt = ts_ // P  # T-tiles per core slice (phase 0)
    h_loc = 2 * HEAD_DIM  # 256 attn-out rows / W_o cols per core
    QCH = 256  # W_dq streaming chunk (phase-0 matmul N)
    half = D_R // 2

    inp = lambda name, shape, dt=F32: nc.declare_dram_parameter(name, shape, dt, isOutput=False)[:]
    x_s = inp("x_s", [ts_, DIM])
    w_dq = inp("w_dq", [DIM, Q_C], F32R)
    w_dkv = inp("w_dkv", [DIM, KV_C], F32R)
    w_kr = inp("w_kr", [DIM, D_R], F32R)
    w_uq = inp("w_uq", [Q_C, h_loc], F32R)
    w_qr = inp("w_qr", [Q_C, P], F32R)  # 2 heads x 64, de-interleaved per head
    w_uk = inp("w_uk", [KV_C, h_loc], F32R)
    w_uv = inp("w_uv", [KV_C, h_loc], F32R)
    w_o = inp("w_o", [DIM, h_loc], F32R)
    cs2 = inp("cs2", [P, t])      # rows [c, s, c, s] (32 each), transposed
    sc2 = inp("sc2", [P, t])      # rows [s, c, s, c]
    cs_s2 = inp("cs_s2", [D_R, ts_])  # per-core slice, rows [c, s]
    sc_s2 = inp("sc_s2", [D_R, ts_])  # rows [s, c]
    rope_a = inp("rope_a", [P, P], F32R)  # blockdiag(A^T, A^T), A=[[I,-I],[0,0]]
    rope_b = inp("rope_b", [P, P], F32R)  # blockdiag(B^T, B^T), B=[[0,0],[I,I]]
    dmask = inp("dmask", [n_mask, P, 512])
    y_s = nc.declare_dram_parameter("y_s", [t, h_loc], F32, isOutput=True)[:]

    AGROWS = Q_C + KV_C + D_R  # 2112
    rg = [list(range(N_CORES))]

    with tile.TileContext(nc) as tc:
        with (
            tc.tile_pool(name="dram", bufs=1, space="DRAM") as dram,
            tc.tile_pool(name="const", bufs=1) as const,
        ):
            agin1 = dram.tile([AGROWS, ts_], F32R)
            agout1 = dram.tile([N_CORES, AGROWS, ts_], F32R)
            agin2 = dram.tile([h_loc, t], F32R)
            agout2 = dram.tile([N_CORES * h_loc, t], F32R)

            ident = const.tile([P, P], F32)
            make_identity(nc, ident)
            ident_r = const.tile([P, P], F32R)
            nc.vector.tensor_copy(ident_r, ident)
            ones_f = const.tile([P, 1], F32)
            nc.vector.memset(ones_f, 1.0)
            ones = const.tile([P, 1], F32R)
            nc.vector.tensor_copy(ones, ones_f)
            ones1 = const.tile([1, P], F32)
            nc.vector.memset(ones1, 1.0)
            epsq = const.tile([P, 1], F32)
            nc.vector.memset(epsq, EPS)
            dm_sb = const.tile([P, n_mask, 512], F32)
            nc.sync.dma_start(dm_sb, dmask.rearrange("u p q -> p u q"))
            cs2_sb = const.tile([P, t], F32, tag="cs2")
            sc2_sb = const.tile([P, t], F32, tag="sc2")
            nc.sync.dma_start(cs2_sb, cs2)
            nc.sync.dma_start(sc2_sb, sc2)
            css_sb = const.tile([D_R, ts_], F32, tag="coss")
            sns_sb = const.tile([D_R, ts_], F32, tag="sins")
            nc.sync.dma_start(css_sb, cs_s2)
            nc.sync.dma_start(sns_sb, sc_s2)
            ra_sb = const.tile([P, P], F32R, tag="ra")
            rb_sb = const.tile([P, P], F32R, tag="rb")
            nc.sync.dma_start(ra_sb, rope_a)
            nc.sync.dma_start(rb_sb, rope_b)

            # ---------------- phase 0 ----------------
            with (
                tc.tile_pool(name="p0", bufs=2) as p0,
                tc.tile_pool(name="p0w", bufs=2) as p0w,
                tc.tile_pool(name="p0s", bufs=1) as p0s,
                tc.tile_pool(name="ps0tp", bufs=2, space="PSUM") as ps0tp,
                tc.tile_pool(name="ps0mm", bufs=2, space="PSUM") as ps0mm,
                tc.tile_pool(name="ps0kr", bufs=1, space="PSUM") as ps0kr,
            ):
                # load x slice and transpose -> xT [P, DIM/P, ts_]
                xt = p0s.tile([P, DIM // P, ts_], F32R, tag="xt")
                for tt in range(ntt):
                    xrow = p0.tile([P, DIM], F32, tag="xrow")
                    nc.sync.dma_start(xrow, x_s[ts(tt, P), :])
                    for c in range(DIM // P):
                        pst = ps0tp.tile([P, P], F32, tag="tp")
                        nc.tensor.matmul(pst, xrow[:, ts(c, P)], ident, start=True, stop=True)
                        nc.vector.tensor_copy(xt[:, c, ts(tt, P)], pst)

                # cq = x @ W_dq -> [P, ntt, Q_C]
                cq = p0s.tile([P, ntt, Q_C], F32R, tag="cq")
                for qcc in range(Q_C // QCH):
                    wblk = p0w.tile([P, DIM // P, QCH], F32R, tag="wdq")
                    nc.sync.dma_start(
                        wblk,
                        w_dq.rearrange("(c p) q -> p c q", p=P)[:, :, ds(qcc * QCH, QCH)],
                    )
                    for tt in range(ntt):
                        psf = ps0mm.tile([P, 512], F32, tag="mm", name="mmq")
                        ps = psf[:, :QCH]
                        for c in range(DIM // P):
                            nc.tensor.matmul(
                                ps,
                                r(xt[:, c, ts(tt, P)]),
                                r(wblk[:, c, :]),
                                start=(c == 0),
                                stop=(c == DIM // P - 1),
                            )
                        nc.vector.tensor_copy(cq[:, tt, ds(qcc * QCH, QCH)], ps)

                # ckv = x @ W_dkv -> [P, ntt, KV_C]
                ckv = p0s.tile([P, ntt, KV_C], F32R, tag="ckv")
                wkv = p0s.tile([P, DIM // P, KV_C], F32R, tag="wdkv")
                nc.sync.dma_start(wkv, w_dkv.rearrange("(c p) q -> p c q", p=P))
                for tt in range(ntt):
                    ps = ps0mm.tile([P, 512], F32, tag="mm")
                    for c in range(DIM // P):
                        nc.tensor.matmul(
                            ps,
                            r(xt[:, c, ts(tt, P)]),
                            r(wkv[:, c, :]),
                            start=(c == 0),
                            stop=(c == DIM // P - 1),
                        )
                    nc.vector.tensor_copy(ckv[:, tt, :], ps)

                # rmsnorm rows (g folded into up-proj weights on host)
                def rms_scale(buf, width):
                    for tt in range(ntt):
                        sq = p0.tile([P, width], F32, tag=f"sq{width}")
                        nc.vector.tensor_mul(sq, buf[:, tt, :], buf[:, tt, :])
                        sos = p0.tile([P, 1], F32, tag=f"sos{width}")
                        nc.vector.reduce_sum(out=sos, in_=sq, axis=mybir.AxisListType.X)
                        nc.scalar.activation(
                            out=sos,
                            in_=sos,
                            func=mybir.ActivationFunctionType.Sqrt,
                            bias=epsq,
                            scale=1.0 / width,
                        )
                        nc.vector.reciprocal(out=sos, in_=sos)
                        nc.vector.tensor_scalar_mul(buf[:, tt, :], buf[:, tt, :], sos)

                rms_scale(cq, Q_C)
                rms_scale(ckv, KV_C)

                # transpose normed latents into agin1 rows
                def store_t(buf, width, row0):
                    for cc in range(width // P):
                        stg = p0.tile([P, ts_], F32R, tag="stg")
                        for tt in range(ntt):
                            pst = ps0tp.tile([P, P], F32, tag="tpr", name="pst")
                            nc.tensor.matmul(pst, buf[:, tt, ts(cc, P)], ident_r, start=True, stop=True)
                            nc.vector.tensor_copy(stg[:, ts(tt, P)], pst)
                        nc.sync.dma_start(agin1[ds(row0 + cc * P, P), :], stg)

                store_t(cq, Q_C, 0)
                store_t(ckv, KV_C, Q_C)

                # k_rope^T = rope((x @ W_kr)^T)  [D_R, ts_]
                wkr = p0s.tile([P, DIM // P, D_R], F32R, tag="wkr")
                nc.sync.dma_start(wkr, w_kr.rearrange("(c p) d -> p c d", p=P))
                pkr = ps0kr.tile([D_R, ts_], F32, tag="krps")
                for c in range(DIM // P):
                    nc.tensor.matmul(
                        pkr,
                        r(wkr[:, c, :]),
                        r(xt[:, c, :]),
                        start=(c == 0),
                        stop=(c == DIM // P - 1),
                    )
                # rope via aligned hadamards + PE combine:
                # ta=[e*c; o*s], tb=[e*s; o*c]; out = A@ta + B@tb
                ta = p0.tile([D_R, ts_], F32R, tag="krta")
                tb = p0.tile([D_R, ts_], F32R, tag="krtb")
                nc.vector.tensor_mul(ta, pkr, css_sb)
                nc.vector.tensor_mul(tb, pkr, sns_sb)
                pkr2 = ps0kr.tile([D_R, ts_], F32, tag="krps2")
                nc.tensor.matmul(
                    pkr2, r(ra_sb[:D_R, :D_R]), r(ta), start=True, stop=False
                )
                nc.tensor.matmul(
                    pkr2, r(rb_sb[:D_R, :D_R]), r(tb), start=False, stop=True
                )
                krs = p0.tile([D_R, ts_], F32R, tag="krs")
                nc.vector.tensor_copy(krs, pkr2)
                nc.sync.dma_start(agin1[ds(Q_C + KV_C, D_R), :], krs)

            nc.gpsimd.collective_compute(
                "AllGather",
                mybir.AluOpType.bypass,
                ins=[agin1[:].opt()],
                outs=[agout1[:].opt()],
                replica_groups=rg,
            )

            # ---------------- phase 1 ----------------
            with (
                tc.tile_pool(name="p1w", bufs=1) as p1w,
                tc.tile_pool(name="p1s", bufs=1) as p1s,
                tc.tile_pool(name="p1", bufs=2) as p1,
            ):
                wuq = p1w.tile([P, Q_C // P, h_loc], F32R, tag="wuq")
                nc.sync.dma_start(wuq, w_uq.rearrange("(c p) d -> p c d", p=P))
                wqr = p1w.tile([P, Q_C // P, P], F32R, tag="wqr")
                nc.sync.dma_start(wqr, w_qr.rearrange("(c p) d -> p c d", p=P))
                wuk = p1w.tile([P, KV_C // P, h_loc], F32R, tag="wuk")
                nc.sync.dma_start(wuk, w_uk.rearrange("(c p) d -> p c d", p=P))
                wuv = p1w.tile([P, KV_C // P, h_loc], F32R, tag="wuv")
                nc.sync.dma_start(wuv, w_uv.rearrange("(c p) d -> p c d", p=P))

                kvc = p1s.tile([P, KV_C // P, t], F32R, tag="kvc")
                krt2 = p1s.tile([P, t], F32R, tag="krt2")
                for rr in range(N_CORES):
                    nc.sync.dma_start(
                        kvc[:, :, ds(rr * ts_, ts_)],
                        agout1[rr, ds(Q_C, KV_C), :].rearrange("(c p) t -> p c t", p=P),
                    )
                    # duplicated rope rows: head A uses 0:64, head B 64:128
                    nc.sync.dma_start(
                        krt2[:D_R, ds(rr * ts_, ts_)],
                        agout1[rr, ds(Q_C + KV_C, D_R), :],
                    )
                    nc.sync.dma_start(
                        krt2[D_R:, ds(rr * ts_, ts_)],
                        agout1[rr, ds(Q_C + KV_C, D_R), :],
                    )

                qt = p1s.tile([P, 2, t], F32R, tag="qt")
                qrt = p1s.tile([P, t], F32R, tag="qrt")
                kt_sb = p1s.tile([P, 2, t], F32R, tag="kt")
                with tc.tile_pool(name="ps1", bufs=4, space="PSUM") as ps1:
                    NQH = Q_C // P // 2  # 6 chunks per half
                    for rr in range(N_CORES):
                        qps = [
                            ps1.tile([P, ts_], F32, tag="dec", name=f"qps{j}")
                            for j in range(3)
                        ]
                        for half_i in range(2):
                            qcb = p1.tile([P, NQH, ts_], F32R, tag="qcb")
                            nc.sync.dma_start(
                                qcb,
                                agout1[rr, ds(half_i * NQH * P, NQH * P), :].rearrange(
                                    "(c p) t -> p c t", p=P
                                ),
                            )
                            for c in range(NQH):
                                cg = half_i * NQH + c
                                first = cg == 0
                                last = cg == Q_C // P - 1
                                for h in range(2):
                                    nc.tensor.matmul(
                                        qps[h],
                                        r(wuq[:, cg, ts(h, P)]),
                                        r(qcb[:, c, :]),
                                        start=first,
                                        stop=last,
                                    )
                                nc.tensor.matmul(
                                    qps[2],
                                    r(wqr[:, cg, :]),
                                    r(qcb[:, c, :]),
                                    start=first,
                                    stop=last,
                                )
                        for h in range(2):
                            nc.vector.tensor_copy(qt[:, h, ds(rr * ts_, ts_)], qps[h])
                        # rope on the raw branch (rows [Ae,Ao,Be,Bo]):
                        # aligned hadamards + per-head PE combine
                        ta_c = p1.tile([P, ts_], F32R, tag="ta")
                        tb_c = p1.tile([P, ts_], F32R, tag="tb")
                        nc.vector.tensor_mul(
                            ta_c, qps[2], cs2_sb[:, ds(rr * ts_, ts_)]
                        )
                        nc.vector.tensor_mul(
                            tb_c, qps[2], sc2_sb[:, ds(rr * ts_, ts_)]
                        )
                        pr = ps1.tile([P, ts_], F32, tag="ropeps", name="ropeps")
                        nc.tensor.matmul(pr, r(ra_sb), r(ta_c), start=True, stop=False)
                        nc.tensor.matmul(pr, r(rb_sb), r(tb_c), start=False, stop=True)
                        nc.vector.tensor_copy(qrt[:, ds(rr * ts_, ts_)], pr)
                        for h in range(2):
                            ps = ps1.tile([P, ts_], F32, tag="dec")
                            for c in range(KV_C // P):
                                nc.tensor.matmul(
                                    ps,
                                    r(wuk[:, c, ts(h, P)]),
                                    r(kvc[:, c, ds(rr * ts_, ts_)]),
                                    start=(c == 0),
                                    stop=(c == KV_C // P - 1),
                                )
                            nc.vector.tensor_copy(kt_sb[:, h, ds(rr * ts_, ts_)], ps)

                    v_sb = p1s.tile([P, t // P, h_loc], F32R, tag="v")
                    for tt in range(t // P):
                        ps = ps1.tile([P, h_loc], F32, tag="dec", name="decv")
                        for c in range(KV_C // P):
                            nc.tensor.matmul(
                                ps,
                                r(kvc[:, c, ts(tt, P)]),
                                r(wuv[:, c, :]),
                                start=(c == 0),
                                stop=(c == KV_C // P - 1),
                            )
                        nc.vector.tensor_copy(v_sb[:, tt, :], ps)

                # attention in S^T [k, q] layout
                with (
                    tc.tile_pool(name="att", bufs=3) as att,
                    tc.tile_pool(name="attp", bufs=3, space="PSUM") as attp,
                    tc.tile_pool(name="attl", bufs=2, space="PSUM") as attl,
                    tc.tile_pool(name="attb", bufs=1, space="PSUM") as attb,
                ):
                    for h in range(2):
                        for qc in range(t // 512):
                            q0 = qc * 512
                            kts = [
                                kt
                                for kt in range(t // P)
                                if block_table[(qc, kt)][0] != "skip"
                            ]
                            avp = attl.tile([P, 512], F32, tag="av")
                            lsp = attl.tile([1, 512], F32, tag="ls")
                            for i, kt in enumerate(kts):
                                sp = attp.tile([P, 512], F32, tag="s")
                                nc.tensor.matmul(
                                    sp,
                                    r(kt_sb[:, h, ts(kt, P)]),
                                    r(qt[:, h, ds(q0, 512)]),
                                    start=True,
                                    stop=False,
                                )
                                nc.tensor.matmul(
                                    sp,
                                    r(krt2[ds(h * D_R, D_R), ts(kt, P)]),
                                    r(qrt[ds(h * D_R, D_R), ds(q0, 512)]),
                                    start=False,
                                    stop=True,
                                    tile_position=(h * D_R, 0),
                                )
                                kind, mi = block_table[(qc, kt)]
                                if kind == "mask":
                                    nc.vector.tensor_add(sp, sp, dm_sb[:, mi, :])
                                es = att.tile([P, 512], F32R, tag="es")
                                nc.scalar.activation(
                                    out=es, in_=sp, func=mybir.ActivationFunctionType.Exp
                                )
                                first, last = i == 0, i == len(kts) - 1
                                nc.tensor.matmul(lsp, r(ones), r(es), start=first, stop=last)
                                nc.tensor.matmul(
                                    avp,
                                    r(v_sb[:, kt, ts(h, P)]),
                                    r(es),
                                    start=first,
                                    stop=last,
                                )
                            rcp = att.tile([1, 512], F32, tag="rcp")
                            nc.vector.reciprocal(out=rcp, in_=lsp)
                            rbp = attb.tile([P, 512], F32, tag="rb")
                            nc.tensor.matmul(rbp, ones1, rcp, start=True, stop=True)
                            rbs = att.tile([P, 512], F32, tag="rbs")
                            nc.vector.tensor_copy(rbs, rbp)
                            ot = att.tile([P, 512], F32R, tag="ot")
                            nc.vector.tensor_mul(ot, avp, rbs)
                            nc.sync.dma_start(agin2[ts(h, P), ds(q0, 512)], ot)

            nc.gpsimd.collective_compute(
                "AllGather",
                mybir.AluOpType.bypass,
                ins=[agin2[:].opt()],
                outs=[agout2[:].opt()],
                replica_groups=rg,
            )

            # ---------------- phase 2 ----------------
            with (
                tc.tile_pool(name="p2w", bufs=1) as p2w,
                tc.tile_pool(name="p2", bufs=2) as p2,
                tc.tile_pool(name="ps2", bufs=4, space="PSUM") as ps2,
            ):
                wo = p2w.tile([P, DIM // P, h_loc], F32R, tag="wo")
                nc.sync.dma_start(wo, w_o.rearrange("(c p) d -> p c d", p=P))
                for tq in range(t // 512):
                    ab = p2.tile([P, DIM // P, 512], F32R, tag="ab")
                    nc.sync.dma_start(
                        ab,
                        agout2.rearrange("(c p) t -> p c t", p=P)[:, :, ds(tq * 512, 512)],
                    )
                    for tt in range(4):
                        ps = ps2.tile([P, h_loc], F32, tag="yps")
                        for c in range(DIM // P):
                            nc.tensor.matmul(
                                ps,
                                r(ab[:, c, ts(tt, P)]),
                                r(wo[:, c, :]),
                                start=(c == 0),
                                stop=(c == DIM // P - 1),
                            )
                        yt = p2.tile([P, h_loc], F32, tag="yt")
                        nc.vector.tensor_copy(yt, ps)
                        nc.sync.dma_start(y_s[ds(tq * 512 + tt * P, P), :], yt)

    nc.compile()
    return nc


_CACHE = {}


def _prep_inputs(x, W_dq, W_uq, W_dkv, W_uk, W_uv, W_qr, W_kr, W_o, g_q, g_kv,
                 freqs_cos, freqs_sin, mask, t):
    scale = 1.0 / math.sqrt(HEAD_DIM + D_R)
    ts_ = t // N_CORES
    x2 = np.ascontiguousarray(np.asarray(x, np.float32).reshape(t, DIM))
    g_q = np.asarray(g_q, np.float32)
    g_kv = np.asarray(g_kv, np.float32)
    wuq_f = np.asarray(W_uq, np.float32) * g_q[:, None] * scale
    wqr_f = np.asarray(W_qr, np.float32) * g_q[:, None] * scale
    wuk_f = np.asarray(W_uk, np.float32) * g_kv[:, None]
    wuv_f = np.asarray(W_uv, np.float32) * g_kv[:, None]
    wkr = np.asarray(W_kr, np.float32)
    # de-interleave rope pairs: [e0..e31, o0..o31]
    perm = np.concatenate([np.arange(0, D_R, 2), np.arange(1, D_R, 2)])
    wkr_p = np.ascontiguousarray(wkr[:, perm])
    cos_t = np.ascontiguousarray(np.asarray(freqs_cos, np.float32).T)  # [32, t]
    sin_t = np.ascontiguousarray(np.asarray(freqs_sin, np.float32).T)
    cs2 = np.concatenate([cos_t, sin_t, cos_t, sin_t], axis=0)  # [128, t]
    sc2 = np.concatenate([sin_t, cos_t, sin_t, cos_t], axis=0)
    half = D_R // 2
    I32 = np.eye(half, dtype=np.float32)
    Z = np.zeros((half, half), np.float32)
    A = np.block([[I32, -I32], [Z, Z]]).astype(np.float32)
    Bm = np.block([[Z, Z], [I32, I32]]).astype(np.float32)
    zz = np.zeros((D_R, D_R), np.float32)
    rope_a = np.ascontiguousarray(np.block([[A.T, zz], [zz, A.T]]))  # [128, 128]
    rope_b = np.ascontiguousarray(np.block([[Bm.T, zz], [zz, Bm.T]]))
    block_table, uniq = _build_block_table(np.asarray(mask, np.float32), t)

    in_maps = []
    for i in range(N_CORES):
        hs = slice(2 * i * HEAD_DIM, (2 * i + 2) * HEAD_DIM)
        qr_cols = [wqr_f[:, 2 * i * D_R + perm], wqr_f[:, (2 * i + 1) * D_R + perm]]
        im = {
            "x_s": np.ascontiguousarray(x2[i * ts_ : (i + 1) * ts_]),
            "w_dq": np.ascontiguousarray(np.asarray(W_dq, np.float32)),
            "w_dkv": np.ascontiguousarray(np.asarray(W_dkv, np.float32)),
            "w_kr": wkr_p,
            "w_uq": np.ascontiguousarray(wuq_f[:, hs]),
            "w_qr": np.ascontiguousarray(np.concatenate(qr_cols, axis=1)),
            "w_uk": np.ascontiguousarray(wuk_f[:, hs]),
            "w_uv": np.ascontiguousarray(wuv_f[:, hs]),
            "w_o": np.ascontiguousarray(np.asarray(W_o, np.float32)[:, hs]),
            "cs2": cs2,
            "sc2": sc2,
            "cs_s2": np.ascontiguousarray(
                np.concatenate([cos_t, sin_t], axis=0)[:, i * ts_ : (i + 1) * ts_]
            ),
            "sc_s2": np.ascontiguousarray(
                np.concatenate([sin_t, cos_t], axis=0)[:, i * ts_ : (i + 1) * ts_]
            ),
            "rope_a": rope_a,
            "rope_b": rope_b,
            "dmask": uniq,
        }
        in_maps.append(im)
    return in_maps, block_table, uniq.shape[0]


def _table_key(t, block_table):
    return (t, tuple(sorted(block_table.items())))


def build_for(t, mask):
    block_table, uniq = _build_block_table(np.asarray(mask, np.float32), t)
    key = _table_key(t, block_table)
    if key not in _CACHE:
        _CACHE[key] = build_program(t, block_table, uniq.shape[0])
    return _CACHE[key]


def kernel(**inputs):
    from concourse.bass_utils import run_bass_kernel_spmd

    t = int(np.asarray(inputs["x"]).shape[1])
    in_maps, block_table, n_mask = _prep_inputs(
        inputs["x"], inputs["W_dq"], inputs["W_uq"], inputs["W_dkv"],
        inputs["W_uk"], inputs["W_uv"], inputs["W_qr"], inputs["W_kr"],
        inputs["W_o"], inputs["g_q"], inputs["g_kv"],
        inputs["freqs_cos"], inputs["freqs_sin"], inputs["mask"], t,
    )
    key = _table_key(t, block_table)
    if key not in _CACHE:
        _CACHE[key] = build_program(t, block_table, n_mask)
    nc = _CACHE[key]
    res = run_bass_kernel_spmd(nc, in_maps, list(range(N_CORES))).results
    y = np.concatenate([res[i]["y_s"] for i in range(N_CORES)], axis=1)
    return y.reshape(B, t, DIM).astype(np.float32)
